# revision 47
# baseline (speedup 1.0000x reference)
"""Exaone4 attention kernel for 8 Trainium2 NeuronCores.

Sharding: tensor-parallel over heads (TP=8). Core i owns query heads
4i..4i+3 and kv head i (one GQA group), processes both batch elements,
and computes a row-parallel partial of the output projection; the host
sums the 8 partials.

Pipeline (fp16 wire dtypes, fp32 PSUM/stats):
  per batch b:
    QKV: single pass over the contraction (6 psum banks: q0..q3,k,v),
         epilogue does RMSNorm via ones-matmul + broadcast-rsqrt and
         RoPE via partition-offset DVE ops (sign folded into the sin
         table, norm weights into per-partition scalars). V transposed
         to [tok, d] tiles with DMA xbar transposes.
    attention: per (chunk, head): scores/PV matmuls depth-4 pipelined;
         exp on ACT (bias -10, sliced to the unmasked region); softmax
         denominator accumulated on DVE and reduced with one
         ones-matmul; 1/sum via DVE fast reciprocal. Outputs stay in
         SBUF (no DRAM scratch).
  out-projection: reads attention outputs straight from SBUF, writes
         fp16 partials; host sums in fp32.

Shapes (hardcoded): B=2, S=2048, H=4096, NH=32, NKV=8, D=128,
WINDOW=1024, eps=1e-5, theta=10000.
"""

import os
import sys

for _p in ("/opt/trn_rl_repo",):
    if _p not in sys.path and os.path.isdir(_p):
        sys.path.insert(0, _p)

import numpy as np

B, S, H = 2, 2048, 4096
NH, NKV, D = 32, 8, 128
WINDOW = 1024
EPS = 1e-5
THETA = 10000.0

NCORES = 8
HPC = NH // NCORES          # query heads per core = 4
QW = HPC * D                # q-proj cols per core = 512
CH = 512                    # sequence chunk
NSC = S // CH               # 4 chunks
HC = H // 128               # 32 contraction chunks
NEG = -1.0e30
EXPB = -10.0                # exp bias so fp16 probs never overflow

_CACHE = {}


def _build():
    import concourse.bass as bass
    import concourse.tile as tile
    from concourse import mybir, bacc

    F32 = mybir.dt.float32
    F16 = mybir.dt.float16
    EXP = mybir.ActivationFunctionType.Exp
    SQUARE = mybir.ActivationFunctionType.Square
    RSQRT = mybir.ActivationFunctionType.Abs_reciprocal_sqrt
    MULT = mybir.AluOpType.mult

    nc = bacc.Bacc("TRN2", target_bir_lowering=False, debug=False)

    hsT = nc.dram_tensor("hsT", [B, H, S], F16, kind="ExternalInput")
    wq_s = nc.dram_tensor("wq_s", [H, QW], F16, kind="ExternalInput")
    wk_s = nc.dram_tensor("wk_s", [H, D], F16, kind="ExternalInput")
    wv_s = nc.dram_tensor("wv_s", [H, D], F16, kind="ExternalInput")
    wo_s = nc.dram_tensor("wo_s", [QW, H], F16, kind="ExternalInput")
    cosT = nc.dram_tensor("cosT", [D, S], F16, kind="ExternalInput")
    sinT = nc.dram_tensor("sinT", [D, S], F16, kind="ExternalInput")
    wcols_d = nc.dram_tensor("wcols", [D, 4], F32, kind="ExternalInput")
    tri_c = nc.dram_tensor("tri_c", [128, 128], F16, kind="ExternalInput")
    tri_w = nc.dram_tensor("tri_w", [128, 128], F16, kind="ExternalInput")
    out_part = nc.dram_tensor("out_part", [B, S, H], F16, kind="ExternalOutput")

    with tile.TileContext(nc) as tc, \
         nc.allow_low_precision(reason="deliberate fp16 matmul pipeline"):
        with tc.tile_pool(name="consts", bufs=1) as consts:
            # Preamble DMA order is tuned for warmup: the sync queue holds
            # only what the first matmuls need (wq chunk 0, wk, wv) before
            # the hs tiles; everything else rides the ACT queue in
            # deadline order (wq chunks 1-3, rope tables, tris, wo).
            # ~128KB per dma: consecutive dmas from one engine land on
            # different DMA rings, so fine splits transfer in parallel
            # (a single dma is capped at ~20GB/s on one ring). The sync
            # queue carries only what gates the first matmuls (wq hc0-15)
            # so the hs tiles right behind it start early; the rest rides
            # the ACT queue in deadline order.
            wq_sb = consts.tile([128, HC, QW], F16)
            for o in range(8):
                nc.sync.dma_start(
                    wq_sb[:, o, :], wq_s.ap()[128 * o:128 * (o + 1), :])
            for o in range(4):
                nc.sync.dma_start(
                    wq_sb[:, 8 + 2 * o:10 + 2 * o, :],
                    wq_s.ap()[1024 + 256 * o:1024 + 256 * (o + 1),
                              :].rearrange("(o p) c -> p o c", p=128))
            for o in range(8):
                nc.scalar.dma_start(
                    wq_sb[:, 16 + 2 * o:18 + 2 * o, :],
                    wq_s.ap()[2048 + 256 * o:2048 + 256 * (o + 1),
                              :].rearrange("(o p) c -> p o c", p=128))
            wk_sb = consts.tile([128, HC, D], F16)
            wv_sb = consts.tile([128, HC, D], F16)
            for o in range(4):
                nc.scalar.dma_start(
                    wk_sb[:, 8 * o:8 * (o + 1), :],
                    wk_s.ap()[1024 * o:1024 * (o + 1), :].rearrange(
                        "(o p) c -> p o c", p=128))
                nc.scalar.dma_start(
                    wv_sb[:, 8 * o:8 * (o + 1), :],
                    wv_s.ap()[1024 * o:1024 * (o + 1), :].rearrange(
                        "(o p) c -> p o c", p=128))
            wcols = consts.tile([D, 4], F32)
            nc.scalar.dma_start(wcols, wcols_d.ap())
            wqcos, wqsin = wcols[:, 0:1], wcols[:, 1:2]
            wkcos, wksin = wcols[:, 2:3], wcols[:, 3:4]
            cos_sb = consts.tile([D, S], F16)
            sin_sb = consts.tile([D, S], F16)
            for o in range(2):
                nc.scalar.dma_start(cos_sb[:, 1024 * o:1024 * (o + 1)],
                                    cosT.ap()[:, 1024 * o:1024 * (o + 1)])
                nc.scalar.dma_start(sin_sb[:, 1024 * o:1024 * (o + 1)],
                                    sinT.ap()[:, 1024 * o:1024 * (o + 1)])
            mc = consts.tile([128, 128], F16)
            nc.scalar.dma_start(mc, tri_c.ap())
            mw = consts.tile([128, 128], F16)
            nc.scalar.dma_start(mw, tri_w.ap())
            wo_sb = consts.tile([128, QW // 128, H], F16)
            for o in range(4):
                nc.scalar.dma_start(
                    wo_sb[:, o, :], wo_s.ap()[128 * o:128 * (o + 1), :])
            ones_k = consts.tile([128, 128], F16)
            nc.vector.memset(ones_k, 1.0)
            expb = consts.tile([128, 1], F32)
            nc.vector.memset(expb, EXPB)
            bias_q = consts.tile([128, 1], F32)
            nc.vector.memset(bias_q, float(D) * EPS)
            bias_k = consts.tile([128, 1], F32)
            nc.vector.memset(bias_k, EPS)

            # attention outputs, SBUF-resident until the out-projection
            scr = consts.tile([128, B, HPC, S], F16)
            # per-batch q/k/v (reused across b)
            qf = consts.tile([128, HPC, S], F16)
            k_full = consts.tile([128, S], F16)
            v_full = consts.tile([128, S // 128, 128], F16)

            # per-group norm blocks: (psum slot, kind, head idx)
            grp_norm = [
                [(0, "q", 0), (1, "q", 1), (2, "q", 2)],
                [(0, "q", 3), (1, "k", 0)],          # slot 2 of grp1 is V
            ]

            for b in range(B):
                # ---------------- QKV + norm + rope ----------------------
                # Two 3-block groups per chunk (q0q1q2 | q3,k,v), psum
                # bufs=2 so a group's epilogue overlaps the next group's
                # matmul stream. The epilogue's PE ops (ssq matmuls) are
                # deferred into the next group's stream so PE never
                # stalls on the ACT square pass.
                with tc.tile_pool(name="hs", bufs=8) as hsp, \
                     tc.tile_pool(name="epi", bufs=2) as epi, \
                     tc.tile_pool(name="ps_qkv", bufs=2, space="PSUM") as pq, \
                     tc.tile_pool(name="ps_aux", bufs=2, space="PSUM") as pa:

                    def epilogue_part1(sc, grp, qkv_ps, sqs, ssqs):
                        """first two ssq matmuls (aux psum bufs=2); the
                        third waits until a wtil has freed a bank, with
                        matmuls in between to cover the ACT latency."""
                        for i in range(min(2, len(grp_norm[grp]))):
                            ssq = pa.tile([128, CH], F32, tag="aux")
                            nc.tensor.matmul(ssq, ones_k, sqs[i],
                                             start=True, stop=True)
                            ssqs[i] = ssq

                    def epilogue_rest(sc, grp, qkv_ps, sqs, ssqs):
                        """remaining ssq matmuls + rsqrt + normalize + rope
                        for one group."""
                        s0 = CH * sc
                        wtils = {}
                        for i, (blk, kind, hd) in enumerate(grp_norm[grp]):
                            if i in ssqs:
                                continue
                            ssq = pa.tile([128, CH], F32, tag="aux")
                            nc.tensor.matmul(ssq, ones_k, sqs[i],
                                             start=True, stop=True)
                            ssqs[i] = ssq
                        for i, (blk, kind, hd) in enumerate(grp_norm[grp]):
                            # q: rsqrt(ssq + D*eps) = rsqrt(mean+eps)/sqrt(D)
                            # (folds the 1/sqrt(D) score scale into q)
                            wtil = epi.tile([128, CH], F16, tag=f"wtil{i}")
                            if kind == "q":
                                nc.scalar.activation(wtil, ssqs[i], RSQRT,
                                                     bias=bias_q, scale=1.0)
                            else:
                                nc.scalar.activation(wtil, ssqs[i], RSQRT,
                                                     bias=bias_k,
                                                     scale=1.0 / D)
                            wtils[i] = wtil
                        for i, (blk, kind, hd) in enumerate(grp_norm[grp]):
                            qhat = epi.tile([128, CH], F16, tag=f"qhat{i}")
                            nc.vector.tensor_mul(qhat, qkv_ps[:, blk, :],
                                                 wtils[i])
                            wcos = wqcos if kind == "q" else wkcos
                            wsin = wqsin if kind == "q" else wksin
                            dst = (qf[:, hd, s0:s0 + CH] if kind == "q"
                                   else k_full[:, s0:s0 + CH])
                            t1 = epi.tile([128, CH], F16, tag=f"t1_{i}")
                            nc.vector.scalar_tensor_tensor(
                                t1, qhat, wcos, cos_sb[:, s0:s0 + CH],
                                op0=MULT, op1=MULT)
                            # in0/in1 must share a base partition (compiler
                            # constraint); sin[d] == sin[d^64] since the
                            # rope table is [freqs, freqs], so slicing sin
                            # at in0's base yields the right values.
                            t2 = epi.tile([128, CH], F16, tag=f"t2_{i}")
                            nc.vector.scalar_tensor_tensor(
                                t2[0:64, :], qhat[64:128, :], wsin[64:128, :],
                                sin_sb[64:128, s0:s0 + CH],
                                op0=MULT, op1=MULT)
                            nc.vector.scalar_tensor_tensor(
                                t2[64:128, :], qhat[0:64, :], wsin[0:64, :],
                                sin_sb[0:64, s0:s0 + CH],
                                op0=MULT, op1=MULT)
                            nc.vector.tensor_add(dst, t1, t2)

                    pending = None
                    for sc in range(NSC):
                        s0 = CH * sc
                        for grp in range(2):
                            qkv_ps = pq.tile([128, 3, CH], F32, tag="qkv")
                            for hp in range(HC // 2):
                                if hp == 3 and pending is not None:
                                    epilogue_part1(*pending)
                                if hp == 5 and pending is not None:
                                    epilogue_rest(*pending)
                                    pending = None
                                ht = hsp.tile([128, 2, CH], F16, tag="ht")
                                nc.sync.dma_start(
                                    ht,
                                    hsT.ap()[b, 256 * hp:256 * (hp + 1),
                                             s0:s0 + CH].rearrange(
                                        "(o p) c -> p o c", p=128))
                                for sub in range(2):
                                    hc = 2 * hp + sub
                                    for bi in range(3):
                                        blk = 3 * grp + bi
                                        if blk < HPC:
                                            lhs = wq_sb[:, hc,
                                                        128 * blk:
                                                        128 * (blk + 1)]
                                        elif blk == HPC:
                                            lhs = wk_sb[:, hc, :]
                                        else:
                                            lhs = wv_sb[:, hc, :]
                                        nc.tensor.matmul(
                                            qkv_ps[:, bi, :], lhs,
                                            ht[:, sub, :],
                                            start=(hc == 0),
                                            stop=(hc == HC - 1))

                            if grp == 1:
                                # V: cast + DMA xbar transpose to [tok, d]
                                v16 = epi.tile([128, CH], F16, tag="v16")
                                nc.scalar.copy(v16, qkv_ps[:, 2, :])
                                for j in range(CH // 128):
                                    nc.sync.dma_start_transpose(
                                        v_full[:, 4 * sc + j, :],
                                        v16[:, 128 * j:128 * (j + 1)])
                            # ACT squares inline; the rest is deferred into
                            # the next group's matmul stream.
                            sqs = {}
                            for i, (blk, kind, hd) in enumerate(grp_norm[grp]):
                                sq = epi.tile([128, CH], F16, tag=f"sq{i}")
                                nc.scalar.activation(sq, qkv_ps[:, blk, :],
                                                     SQUARE)
                                sqs[i] = sq
                            pending = (sc, grp, qkv_ps, sqs, {})
                    # last group's epilogue before the pool closes
                    epilogue_part1(*pending)
                    epilogue_rest(*pending)
                    pending = None

                # ---------------- attention for batch b ------------------
                # Emission order is engine-order: on DVE the mask adds are
                # kept ahead of the pracc accumulates (else exp_{i+1} chains
                # behind acc_i and the loop serializes), and each head's
                # tail chain (sum/clamp/recip/normalize) is deferred into
                # the next head's early stream.
                # Heads processed in PAIRS: the two score matmuls share a
                # k-tile lhsT and land in one 2-bank psum tile, then one
                # exp / accumulate / sum / reciprocal / normalize covers
                # both heads — halving ACT init overheads and DVE op
                # count. Each pair's tail chain is deferred into the next
                # pair's early stream.
                with tc.tile_pool(name="pr", bufs=4) as prp, \
                     tc.tile_pool(name="acc", bufs=2) as accp, \
                     tc.tile_pool(name="tail", bufs=2) as tlp, \
                     tc.tile_pool(name="ps_s", bufs=3, space="PSUM") as pss, \
                     tc.tile_pool(name="ps_o", bufs=1, space="PSUM") as pso:
                    LOOK = 3
                    pend_tail = None

                    def emit_tail(b_, hp_, s0_, o2_, pracc2_):
                        # sums broadcast to all partitions via the all-ones
                        # lhsT, fast reciprocal, normalize into scr.
                        # no zero-clamp: sum==0 needs all ~1024 scores
                        # below -7.3 sigma, impossible for this data.
                        # one matmul per head: a matmul out cannot span
                        # psum banks (max 512 fp32 free)
                        sum2 = pss.tile([128, 2, CH], F32, tag="s",
                                        name="sum2")
                        for hh in range(2):
                            nc.tensor.matmul(sum2[:, hh, :], ones_k,
                                             pracc2_[:, hh, :],
                                             start=True, stop=True)
                        rq2 = tlp.tile([128, 2, CH], F32, tag="rq")
                        nc.vector.reciprocal_approx_fast(rq2, sum2)
                        nc.vector.tensor_mul(
                            scr[:, b_, 2 * hp_:2 * hp_ + 2, s0_:s0_ + CH],
                            o2_, rq2)

                    for sc in range(NSC):
                        s0 = CH * sc
                        kis = list(range(max(0, 4 * sc - 8), 4 * sc + 4))
                        n = len(kis)
                        for hp in range(HPC // 2):
                            o2 = pso.tile([128, 2, CH], F32, tag="o")
                            pracc2 = accp.tile([128, 2, CH], F16, tag="acc")
                            prs = {}
                            bounds = {}

                            def score(i):
                                ki = kis[i]
                                delta = CH * sc - 128 * ki
                                lo, hi = 0, CH
                                jm = None
                                if delta <= 0:
                                    jm, msk = -delta // 128, mc
                                    lo = 128 * jm
                                elif delta >= 640:
                                    jm, msk = (1024 - delta) // 128, mw
                                    hi = 128 * (jm + 1)
                                # compute only the unmasked q columns; the
                                # bias of -10 bounds exp even on masked
                                # entries, so masking is a cheap fp16
                                # 0/1-triangle multiply after the exp.
                                s2 = pss.tile([128, 2, CH], F32, tag="s")
                                for hh in range(2):
                                    nc.tensor.matmul(
                                        s2[:, hh, lo:hi],
                                        k_full[:, 128 * ki:128 * (ki + 1)],
                                        qf[:, 2 * hp + hh,
                                           s0 + lo:s0 + hi],
                                        start=True, stop=True)
                                pr2 = prp.tile([128, 2, CH], F16, tag="pr")
                                nc.scalar.activation(pr2[:, :, lo:hi],
                                                     s2[:, :, lo:hi], EXP,
                                                     bias=expb)
                                if jm is not None:
                                    for hh in range(2):
                                        sub = pr2[:, hh,
                                                  128 * jm:128 * (jm + 1)]
                                        nc.vector.tensor_mul(sub, sub, msk)
                                for hh in range(2):
                                    if lo > 0:
                                        nc.gpsimd.memset(pr2[:, hh, :lo], 0.0)
                                    if hi < CH:
                                        nc.gpsimd.memset(pr2[:, hh, hi:], 0.0)
                                prs[i] = pr2
                                bounds[i] = (lo, hi)

                            def acc(i):
                                # accumulate only the valid columns (the
                                # rest of pr is zero anyway)
                                lo, hi = bounds[i]
                                if i == 0:
                                    for hh in range(2):
                                        if lo > 0:
                                            nc.gpsimd.memset(
                                                pracc2[:, hh, :lo], 0.0)
                                        if hi < CH:
                                            nc.gpsimd.memset(
                                                pracc2[:, hh, hi:], 0.0)
                                    nc.vector.tensor_copy(
                                        pracc2[:, :, lo:hi],
                                        prs[i][:, :, lo:hi])
                                else:
                                    nc.vector.tensor_add(
                                        pracc2[:, :, lo:hi],
                                        pracc2[:, :, lo:hi],
                                        prs[i][:, :, lo:hi])

                            def pv(i):
                                for hh in range(2):
                                    nc.tensor.matmul(
                                        o2[:, hh, :], v_full[:, kis[i], :],
                                        prs[i][:, hh, :],
                                        start=(i == 0), stop=(i == n - 1))

                            for i in range(min(LOOK, n)):
                                score(i)
                            if pend_tail is not None:
                                emit_tail(*pend_tail)
                                pend_tail = None
                            for i in range(n):
                                if i + LOOK < n:
                                    score(i + LOOK)
                                acc(i)
                                pv(i)
                            pend_tail = (b, hp, s0, o2, pracc2)
                    emit_tail(*pend_tail)
                    pend_tail = None

            # ---------------- output projection ----------------------
            with tc.tile_pool(name="ostg", bufs=4) as ost, \
                 tc.tile_pool(name="ps_c", bufs=1, space="PSUM") as pc:
                NR = QW // 128
                for b in range(B):
                    for st in range(S // 128):
                        for g in range(2):
                            hcbs = range(4 * g, 4 * g + 4)
                            c_tiles = {hcb: pc.tile([128, 512], F32,
                                                    tag=f"c{hcb}",
                                                    name=f"c{hcb}")
                                       for hcb in hcbs}
                            for r in range(NR):
                                a_t = scr[:, b, r, 128 * st:128 * (st + 1)]
                                for hcb in hcbs:
                                    nc.tensor.matmul(
                                        c_tiles[hcb], a_t,
                                        wo_sb[:, r,
                                              512 * hcb:512 * (hcb + 1)],
                                        start=(r == 0), stop=(r == NR - 1))
                            for hcb in hcbs:
                                o_sb = ost.tile([128, 512], F16, tag="ostg")
                                nc.scalar.copy(o_sb, c_tiles[hcb])
                                nc.sync.dma_start(
                                    out_part.ap()[b, 128 * st:128 * (st + 1),
                                                  512 * hcb:512 * (hcb + 1)],
                                    o_sb)

    nc.compile()
    return nc


def _host_prep(hidden_states, wq, wk, wv, wo, q_norm_w, k_norm_w):
    """Build the per-core input maps (fp16 wire dtypes)."""
    f16 = np.float16
    f32 = np.float32
    hsT = np.ascontiguousarray(
        np.transpose(hidden_states, (0, 2, 1))).astype(f16)

    pos = np.arange(S, dtype=np.float64)
    inv_freq = 1.0 / (THETA ** (np.arange(0, D // 2, dtype=np.float64)
                                / (D // 2)))
    freqs = pos[None, :] * inv_freq[:, None]            # [D/2, S]
    emb = np.concatenate([freqs, freqs], axis=0)        # [D, S]
    cosT = np.cos(emb).astype(f16)
    sinT = np.sin(emb).astype(f16)

    # norm weights folded into per-partition rope scalars; the sign of
    # rotate_half folded into wsin (negative for output partitions >= 64,
    # which read input partitions < 64... sign indexed by input partition:
    # wsin[p] multiplies qhat[p] feeding output partition (p+64)%128.
    qw = q_norm_w.astype(f32)
    kw = k_norm_w.astype(f32)
    sgn = np.where(np.arange(D) < 64, 1.0, -1.0).astype(f32)
    wcols = np.stack([qw, qw * sgn, kw, kw * sgn], axis=1)

    kd = np.arange(128)[:, None]
    qd = np.arange(128)[None, :]
    tri_c = (qd >= kd).astype(f16)
    tri_w = (qd < kd).astype(f16)

    common = {
        "hsT": hsT,
        "cosT": cosT,
        "sinT": sinT,
        "wcols": wcols,
        "tri_c": tri_c,
        "tri_w": tri_w,
    }
    in_maps = []
    for c in range(NCORES):
        m = dict(common)
        m["wq_s"] = np.ascontiguousarray(wq[:, QW * c:QW * (c + 1)]).astype(f16)
        m["wk_s"] = np.ascontiguousarray(wk[:, D * c:D * (c + 1)]).astype(f16)
        m["wv_s"] = np.ascontiguousarray(wv[:, D * c:D * (c + 1)]).astype(f16)
        m["wo_s"] = np.ascontiguousarray(wo[QW * c:QW * (c + 1), :]).astype(f16)
        in_maps.append(m)
    return in_maps


def kernel(hidden_states, wq, wk, wv, wo, q_norm_w, k_norm_w,
           _trace=False, _return_results=False):
    from concourse import bass_utils

    hidden_states = np.asarray(hidden_states)
    wq, wk, wv, wo = (np.asarray(a) for a in (wq, wk, wv, wo))
    q_norm_w, k_norm_w = np.asarray(q_norm_w), np.asarray(k_norm_w)

    if "nc" not in _CACHE:
        _CACHE["nc"] = _build()
    nc = _CACHE["nc"]

    in_maps = _host_prep(hidden_states, wq, wk, wv, wo, q_norm_w, k_norm_w)
    res = bass_utils.run_bass_kernel_spmd(
        nc, in_maps, core_ids=list(range(NCORES)), trace=_trace)

    out = np.zeros((B, S, H), np.float32)
    for c in range(NCORES):
        out += res.results[c]["out_part"].astype(np.float32)
    if _return_results:
        return out, res
    return out


# revision 49
# speedup vs baseline: 1.0216x; 1.0216x over previous
"""Exaone4 attention kernel for 8 Trainium2 NeuronCores.

Sharding: tensor-parallel over heads (TP=8). Core i owns query heads
4i..4i+3 and kv head i (one GQA group), processes both batch elements,
and computes a row-parallel partial of the output projection; the host
sums the 8 partials.

Pipeline (fp16 wire dtypes, fp32 PSUM/stats):
  per batch b:
    QKV: single pass over the contraction (6 psum banks: q0..q3,k,v),
         epilogue does RMSNorm via ones-matmul + broadcast-rsqrt and
         RoPE via partition-offset DVE ops (sign folded into the sin
         table, norm weights into per-partition scalars). V transposed
         to [tok, d] tiles with DMA xbar transposes.
    attention: per (chunk, head): scores/PV matmuls depth-4 pipelined;
         exp on ACT (bias -10, sliced to the unmasked region); softmax
         denominator accumulated on DVE and reduced with one
         ones-matmul; 1/sum via DVE fast reciprocal. Outputs stay in
         SBUF (no DRAM scratch).
  out-projection: reads attention outputs straight from SBUF, writes
         fp16 partials; host sums in fp32.

Shapes (hardcoded): B=2, S=2048, H=4096, NH=32, NKV=8, D=128,
WINDOW=1024, eps=1e-5, theta=10000.
"""

import os
import sys

for _p in ("/opt/trn_rl_repo",):
    if _p not in sys.path and os.path.isdir(_p):
        sys.path.insert(0, _p)

import numpy as np

B, S, H = 2, 2048, 4096
NH, NKV, D = 32, 8, 128
WINDOW = 1024
EPS = 1e-5
THETA = 10000.0

NCORES = 8
HPC = NH // NCORES          # query heads per core = 4
QW = HPC * D                # q-proj cols per core = 512
CH = 512                    # sequence chunk
NSC = S // CH               # 4 chunks
HC = H // 128               # 32 contraction chunks
NEG = -1.0e30
EXPB = -10.0                # exp bias so fp16 probs never overflow

_CACHE = {}


def _build():
    import concourse.bass as bass
    import concourse.tile as tile
    from concourse import mybir, bacc

    F32 = mybir.dt.float32
    F16 = mybir.dt.float16
    EXP = mybir.ActivationFunctionType.Exp
    SQUARE = mybir.ActivationFunctionType.Square
    RSQRT = mybir.ActivationFunctionType.Abs_reciprocal_sqrt
    MULT = mybir.AluOpType.mult

    nc = bacc.Bacc("TRN2", target_bir_lowering=False, debug=False)

    hsT = nc.dram_tensor("hsT", [B, H, S], F16, kind="ExternalInput")
    wq_s = nc.dram_tensor("wq_s", [H, QW], F16, kind="ExternalInput")
    wk_s = nc.dram_tensor("wk_s", [H, D], F16, kind="ExternalInput")
    wv_s = nc.dram_tensor("wv_s", [H, D], F16, kind="ExternalInput")
    wo_s = nc.dram_tensor("wo_s", [QW, H], F16, kind="ExternalInput")
    cosT = nc.dram_tensor("cosT", [D, S], F16, kind="ExternalInput")
    sinT = nc.dram_tensor("sinT", [D, S], F16, kind="ExternalInput")
    wcols_d = nc.dram_tensor("wcols", [D, 4], F32, kind="ExternalInput")
    tri_c = nc.dram_tensor("tri_c", [128, 128], F16, kind="ExternalInput")
    tri_w = nc.dram_tensor("tri_w", [128, 128], F16, kind="ExternalInput")
    out_part = nc.dram_tensor("out_part", [B, S, H], F16, kind="ExternalOutput")

    with tile.TileContext(nc) as tc, \
         nc.allow_low_precision(reason="deliberate fp16 matmul pipeline"):
        with tc.tile_pool(name="consts", bufs=1) as consts:
            # Preamble DMA order is tuned for warmup: the sync queue holds
            # only what the first matmuls need (wq chunk 0, wk, wv) before
            # the hs tiles; everything else rides the ACT queue in
            # deadline order (wq chunks 1-3, rope tables, tris, wo).
            # ~128KB per dma: consecutive dmas from one engine land on
            # different DMA rings, so fine splits transfer in parallel
            # (a single dma is capped at ~20GB/s on one ring). The sync
            # queue carries only what gates the first matmuls (wq hc0-15)
            # so the hs tiles right behind it start early; the rest rides
            # the ACT queue in deadline order.
            wq_sb = consts.tile([128, HC, QW], F16)
            for o in range(8):
                nc.sync.dma_start(
                    wq_sb[:, o, :], wq_s.ap()[128 * o:128 * (o + 1), :])
            for o in range(4):
                nc.sync.dma_start(
                    wq_sb[:, 8 + 2 * o:10 + 2 * o, :],
                    wq_s.ap()[1024 + 256 * o:1024 + 256 * (o + 1),
                              :].rearrange("(o p) c -> p o c", p=128))
            for o in range(8):
                nc.scalar.dma_start(
                    wq_sb[:, 16 + 2 * o:18 + 2 * o, :],
                    wq_s.ap()[2048 + 256 * o:2048 + 256 * (o + 1),
                              :].rearrange("(o p) c -> p o c", p=128))
            wk_sb = consts.tile([128, HC, D], F16)
            wv_sb = consts.tile([128, HC, D], F16)
            for o in range(4):
                nc.scalar.dma_start(
                    wk_sb[:, 8 * o:8 * (o + 1), :],
                    wk_s.ap()[1024 * o:1024 * (o + 1), :].rearrange(
                        "(o p) c -> p o c", p=128))
                nc.scalar.dma_start(
                    wv_sb[:, 8 * o:8 * (o + 1), :],
                    wv_s.ap()[1024 * o:1024 * (o + 1), :].rearrange(
                        "(o p) c -> p o c", p=128))
            wcols = consts.tile([D, 4], F32)
            nc.scalar.dma_start(wcols, wcols_d.ap())
            wqcos, wqsin = wcols[:, 0:1], wcols[:, 1:2]
            wkcos, wksin = wcols[:, 2:3], wcols[:, 3:4]
            cos_sb = consts.tile([D, S], F16)
            sin_sb = consts.tile([D, S], F16)
            for o in range(2):
                nc.scalar.dma_start(cos_sb[:, 1024 * o:1024 * (o + 1)],
                                    cosT.ap()[:, 1024 * o:1024 * (o + 1)])
                nc.scalar.dma_start(sin_sb[:, 1024 * o:1024 * (o + 1)],
                                    sinT.ap()[:, 1024 * o:1024 * (o + 1)])
            mc = consts.tile([128, 128], F16)
            nc.scalar.dma_start(mc, tri_c.ap())
            mw = consts.tile([128, 128], F16)
            nc.scalar.dma_start(mw, tri_w.ap())
            wo_sb = consts.tile([128, QW // 128, H], F16)
            for o in range(4):
                nc.scalar.dma_start(
                    wo_sb[:, o, :], wo_s.ap()[128 * o:128 * (o + 1), :])
            ones_k = consts.tile([128, 128], F16)
            nc.vector.memset(ones_k, 1.0)
            expb = consts.tile([128, 1], F32)
            nc.vector.memset(expb, EXPB)
            bias_q = consts.tile([128, 1], F32)
            nc.vector.memset(bias_q, float(D) * EPS)
            bias_k = consts.tile([128, 1], F32)
            nc.vector.memset(bias_k, EPS)

            # attention outputs, SBUF-resident until the out-projection
            scr = consts.tile([128, B, HPC, S], F16)
            # per-batch q/k/v (reused across b)
            qf = consts.tile([128, HPC, S], F16)
            k_full = consts.tile([128, S], F16)
            v_full = consts.tile([128, S // 128, 128], F16)

            # per-group norm blocks: (psum slot, kind, head idx)
            grp_norm = [
                [(0, "q", 0), (1, "q", 1), (2, "q", 2)],
                [(0, "q", 3), (1, "k", 0)],          # slot 2 of grp1 is V
            ]

            for b in range(B):
                # ---------------- QKV + norm + rope ----------------------
                # Two 3-block groups per chunk (q0q1q2 | q3,k,v), psum
                # bufs=2 so a group's epilogue overlaps the next group's
                # matmul stream. The epilogue's PE ops (ssq matmuls) are
                # deferred into the next group's stream so PE never
                # stalls on the ACT square pass.
                with tc.tile_pool(name="hs", bufs=10) as hsp, \
                     tc.tile_pool(name="epi", bufs=2) as epi, \
                     tc.tile_pool(name="ps_qkv", bufs=2, space="PSUM") as pq, \
                     tc.tile_pool(name="ps_aux", bufs=2, space="PSUM") as pa:

                    def epilogue_part1(sc, grp, qkv_ps, sqs, ssqs):
                        """first two ssq matmuls (aux psum bufs=2); the
                        third waits until a wtil has freed a bank, with
                        matmuls in between to cover the ACT latency."""
                        for i in range(min(2, len(grp_norm[grp]))):
                            ssq = pa.tile([128, CH], F32, tag="aux")
                            nc.tensor.matmul(ssq, ones_k, sqs[i],
                                             start=True, stop=True)
                            ssqs[i] = ssq

                    def epilogue_rest(sc, grp, qkv_ps, sqs, ssqs):
                        """remaining ssq matmuls + rsqrt + normalize + rope
                        for one group."""
                        s0 = CH * sc
                        wtils = {}
                        for i, (blk, kind, hd) in enumerate(grp_norm[grp]):
                            if i in ssqs:
                                continue
                            ssq = pa.tile([128, CH], F32, tag="aux")
                            nc.tensor.matmul(ssq, ones_k, sqs[i],
                                             start=True, stop=True)
                            ssqs[i] = ssq
                        for i, (blk, kind, hd) in enumerate(grp_norm[grp]):
                            # q: rsqrt(ssq + D*eps) = rsqrt(mean+eps)/sqrt(D)
                            # (folds the 1/sqrt(D) score scale into q)
                            wtil = epi.tile([128, CH], F16, tag=f"wtil{i}")
                            if kind == "q":
                                nc.scalar.activation(wtil, ssqs[i], RSQRT,
                                                     bias=bias_q, scale=1.0)
                            else:
                                nc.scalar.activation(wtil, ssqs[i], RSQRT,
                                                     bias=bias_k,
                                                     scale=1.0 / D)
                            wtils[i] = wtil
                        for i, (blk, kind, hd) in enumerate(grp_norm[grp]):
                            qhat = epi.tile([128, CH], F16, tag=f"qhat{i}")
                            nc.vector.tensor_mul(qhat, qkv_ps[:, blk, :],
                                                 wtils[i])
                            wcos = wqcos if kind == "q" else wkcos
                            wsin = wqsin if kind == "q" else wksin
                            dst = (qf[:, hd, s0:s0 + CH] if kind == "q"
                                   else k_full[:, s0:s0 + CH])
                            t1 = epi.tile([128, CH], F16, tag=f"t1_{i}")
                            nc.vector.scalar_tensor_tensor(
                                t1, qhat, wcos, cos_sb[:, s0:s0 + CH],
                                op0=MULT, op1=MULT)
                            # in0/in1 must share a base partition (compiler
                            # constraint); sin[d] == sin[d^64] since the
                            # rope table is [freqs, freqs], so slicing sin
                            # at in0's base yields the right values.
                            t2 = epi.tile([128, CH], F16, tag=f"t2_{i}")
                            nc.vector.scalar_tensor_tensor(
                                t2[0:64, :], qhat[64:128, :], wsin[64:128, :],
                                sin_sb[64:128, s0:s0 + CH],
                                op0=MULT, op1=MULT)
                            nc.vector.scalar_tensor_tensor(
                                t2[64:128, :], qhat[0:64, :], wsin[0:64, :],
                                sin_sb[0:64, s0:s0 + CH],
                                op0=MULT, op1=MULT)
                            nc.vector.tensor_add(dst, t1, t2)

                    pending = None
                    for sc in range(NSC):
                        s0 = CH * sc
                        for grp in range(2):
                            qkv_ps = pq.tile([128, 3, CH], F32, tag="qkv")
                            for hp in range(HC // 2):
                                if hp == 3 and pending is not None:
                                    epilogue_part1(*pending)
                                if hp == 5 and pending is not None:
                                    epilogue_rest(*pending)
                                    pending = None
                                ht = hsp.tile([128, 2, CH], F16, tag="ht")
                                nc.sync.dma_start(
                                    ht,
                                    hsT.ap()[b, 256 * hp:256 * (hp + 1),
                                             s0:s0 + CH].rearrange(
                                        "(o p) c -> p o c", p=128))
                                for sub in range(2):
                                    hc = 2 * hp + sub
                                    for bi in range(3):
                                        blk = 3 * grp + bi
                                        if blk < HPC:
                                            lhs = wq_sb[:, hc,
                                                        128 * blk:
                                                        128 * (blk + 1)]
                                        elif blk == HPC:
                                            lhs = wk_sb[:, hc, :]
                                        else:
                                            lhs = wv_sb[:, hc, :]
                                        nc.tensor.matmul(
                                            qkv_ps[:, bi, :], lhs,
                                            ht[:, sub, :],
                                            start=(hc == 0),
                                            stop=(hc == HC - 1))

                            if grp == 1:
                                # V: cast + DMA xbar transpose to [tok, d]
                                v16 = epi.tile([128, CH], F16, tag="v16")
                                nc.scalar.copy(v16, qkv_ps[:, 2, :])
                                for j in range(CH // 128):
                                    nc.sync.dma_start_transpose(
                                        v_full[:, 4 * sc + j, :],
                                        v16[:, 128 * j:128 * (j + 1)])
                            # ACT squares inline; the rest is deferred into
                            # the next group's matmul stream.
                            sqs = {}
                            for i, (blk, kind, hd) in enumerate(grp_norm[grp]):
                                sq = epi.tile([128, CH], F16, tag=f"sq{i}")
                                nc.scalar.activation(sq, qkv_ps[:, blk, :],
                                                     SQUARE)
                                sqs[i] = sq
                            pending = (sc, grp, qkv_ps, sqs, {})
                    # last group's epilogue before the pool closes
                    epilogue_part1(*pending)
                    epilogue_rest(*pending)
                    pending = None

                # ---------------- attention for batch b ------------------
                # Emission order is engine-order: on DVE the mask adds are
                # kept ahead of the pracc accumulates (else exp_{i+1} chains
                # behind acc_i and the loop serializes), and each head's
                # tail chain (sum/clamp/recip/normalize) is deferred into
                # the next head's early stream.
                # Heads processed in PAIRS: the two score matmuls share a
                # k-tile lhsT and land in one 2-bank psum tile, then one
                # exp / accumulate / sum / reciprocal / normalize covers
                # both heads — halving ACT init overheads and DVE op
                # count. Each pair's tail chain is deferred into the next
                # pair's early stream.
                with tc.tile_pool(name="pr", bufs=5) as prp, \
                     tc.tile_pool(name="acc", bufs=2) as accp, \
                     tc.tile_pool(name="tail", bufs=2) as tlp, \
                     tc.tile_pool(name="ps_s", bufs=2, space="PSUM") as pss, \
                     tc.tile_pool(name="ps_o", bufs=1, space="PSUM") as pso, \
                     tc.tile_pool(name="ps_m", bufs=1, space="PSUM") as psm:
                    LOOK = 2
                    pend_tail = None

                    def emit_tail(b_, hp_, s0_, o2_, pracc2_):
                        # sums broadcast to all partitions via the all-ones
                        # lhsT, fast reciprocal, normalize into scr.
                        # no zero-clamp: sum==0 needs all ~1024 scores
                        # below -7.3 sigma, impossible for this data.
                        # one matmul per head: a matmul out cannot span
                        # psum banks (max 512 fp32 free)
                        sum2 = psm.tile([128, 2, CH], F32, tag="sum")
                        for hh in range(2):
                            nc.tensor.matmul(sum2[:, hh, :], ones_k,
                                             pracc2_[:, hh, :],
                                             start=True, stop=True)
                        rq2 = tlp.tile([128, 2, CH], F32, tag="rq")
                        nc.vector.reciprocal_approx_fast(rq2, sum2)
                        nc.vector.tensor_mul(
                            scr[:, b_, 2 * hp_:2 * hp_ + 2, s0_:s0_ + CH],
                            o2_, rq2)

                    for sc in range(NSC):
                        s0 = CH * sc
                        kis = list(range(max(0, 4 * sc - 8), 4 * sc + 4))
                        n = len(kis)
                        for hp in range(HPC // 2):
                            o2 = pso.tile([128, 2, CH], F32, tag="o")
                            pracc2 = accp.tile([128, 2, CH], F16, tag="acc")
                            prs = {}
                            bounds = {}

                            def score(i):
                                ki = kis[i]
                                delta = CH * sc - 128 * ki
                                lo, hi = 0, CH
                                jm = None
                                if delta <= 0:
                                    jm, msk = -delta // 128, mc
                                    lo = 128 * jm
                                elif delta >= 640:
                                    jm, msk = (1024 - delta) // 128, mw
                                    hi = 128 * (jm + 1)
                                # compute only the unmasked q columns; the
                                # bias of -10 bounds exp even on masked
                                # entries, so masking is a cheap fp16
                                # 0/1-triangle multiply after the exp.
                                s2 = pss.tile([128, 2, CH], F32, tag="s")
                                for hh in range(2):
                                    nc.tensor.matmul(
                                        s2[:, hh, lo:hi],
                                        k_full[:, 128 * ki:128 * (ki + 1)],
                                        qf[:, 2 * hp + hh,
                                           s0 + lo:s0 + hi],
                                        start=True, stop=True)
                                pr2 = prp.tile([128, 2, CH], F16, tag="pr")
                                nc.scalar.activation(pr2[:, :, lo:hi],
                                                     s2[:, :, lo:hi], EXP,
                                                     bias=expb)
                                if jm is not None:
                                    for hh in range(2):
                                        sub = pr2[:, hh,
                                                  128 * jm:128 * (jm + 1)]
                                        nc.vector.tensor_mul(sub, sub, msk)
                                for hh in range(2):
                                    if lo > 0:
                                        nc.gpsimd.memset(pr2[:, hh, :lo], 0.0)
                                    if hi < CH:
                                        nc.gpsimd.memset(pr2[:, hh, hi:], 0.0)
                                prs[i] = pr2
                                bounds[i] = (lo, hi)

                            def acc(i):
                                # accumulate only the valid columns (the
                                # rest of pr is zero anyway)
                                lo, hi = bounds[i]
                                if i == 0:
                                    for hh in range(2):
                                        if lo > 0:
                                            nc.gpsimd.memset(
                                                pracc2[:, hh, :lo], 0.0)
                                        if hi < CH:
                                            nc.gpsimd.memset(
                                                pracc2[:, hh, hi:], 0.0)
                                    nc.vector.tensor_copy(
                                        pracc2[:, :, lo:hi],
                                        prs[i][:, :, lo:hi])
                                else:
                                    nc.vector.tensor_add(
                                        pracc2[:, :, lo:hi],
                                        pracc2[:, :, lo:hi],
                                        prs[i][:, :, lo:hi])

                            def pv(i):
                                for hh in range(2):
                                    nc.tensor.matmul(
                                        o2[:, hh, :], v_full[:, kis[i], :],
                                        prs[i][:, hh, :],
                                        start=(i == 0), stop=(i == n - 1))

                            for i in range(min(LOOK, n)):
                                score(i)
                            if pend_tail is not None:
                                emit_tail(*pend_tail)
                                pend_tail = None
                            for i in range(n):
                                if i + LOOK < n:
                                    score(i + LOOK)
                                acc(i)
                                pv(i)
                            pend_tail = (b, hp, s0, o2, pracc2)
                    emit_tail(*pend_tail)
                    pend_tail = None

            # ---------------- output projection ----------------------
            with tc.tile_pool(name="ostg", bufs=6) as ost, \
                 tc.tile_pool(name="ps_c", bufs=1, space="PSUM") as pc:
                NR = QW // 128
                for b in range(B):
                    for st in range(S // 128):
                        for g in range(2):
                            hcbs = range(4 * g, 4 * g + 4)
                            c_tiles = {hcb: pc.tile([128, 512], F32,
                                                    tag=f"c{hcb}",
                                                    name=f"c{hcb}")
                                       for hcb in hcbs}
                            for r in range(NR):
                                a_t = scr[:, b, r, 128 * st:128 * (st + 1)]
                                for hcb in hcbs:
                                    nc.tensor.matmul(
                                        c_tiles[hcb], a_t,
                                        wo_sb[:, r,
                                              512 * hcb:512 * (hcb + 1)],
                                        start=(r == 0), stop=(r == NR - 1))
                            for hcb in hcbs:
                                o_sb = ost.tile([128, 512], F16, tag="ostg")
                                nc.scalar.copy(o_sb, c_tiles[hcb])
                                nc.sync.dma_start(
                                    out_part.ap()[b, 128 * st:128 * (st + 1),
                                                  512 * hcb:512 * (hcb + 1)],
                                    o_sb)

    nc.compile()
    return nc


def _host_prep(hidden_states, wq, wk, wv, wo, q_norm_w, k_norm_w):
    """Build the per-core input maps (fp16 wire dtypes)."""
    f16 = np.float16
    f32 = np.float32
    hsT = np.ascontiguousarray(
        np.transpose(hidden_states, (0, 2, 1))).astype(f16)

    pos = np.arange(S, dtype=np.float64)
    inv_freq = 1.0 / (THETA ** (np.arange(0, D // 2, dtype=np.float64)
                                / (D // 2)))
    freqs = pos[None, :] * inv_freq[:, None]            # [D/2, S]
    emb = np.concatenate([freqs, freqs], axis=0)        # [D, S]
    cosT = np.cos(emb).astype(f16)
    sinT = np.sin(emb).astype(f16)

    # norm weights folded into per-partition rope scalars; the sign of
    # rotate_half folded into wsin (negative for output partitions >= 64,
    # which read input partitions < 64... sign indexed by input partition:
    # wsin[p] multiplies qhat[p] feeding output partition (p+64)%128.
    qw = q_norm_w.astype(f32)
    kw = k_norm_w.astype(f32)
    sgn = np.where(np.arange(D) < 64, 1.0, -1.0).astype(f32)
    wcols = np.stack([qw, qw * sgn, kw, kw * sgn], axis=1)

    kd = np.arange(128)[:, None]
    qd = np.arange(128)[None, :]
    tri_c = (qd >= kd).astype(f16)
    tri_w = (qd < kd).astype(f16)

    common = {
        "hsT": hsT,
        "cosT": cosT,
        "sinT": sinT,
        "wcols": wcols,
        "tri_c": tri_c,
        "tri_w": tri_w,
    }
    in_maps = []
    for c in range(NCORES):
        m = dict(common)
        m["wq_s"] = np.ascontiguousarray(wq[:, QW * c:QW * (c + 1)]).astype(f16)
        m["wk_s"] = np.ascontiguousarray(wk[:, D * c:D * (c + 1)]).astype(f16)
        m["wv_s"] = np.ascontiguousarray(wv[:, D * c:D * (c + 1)]).astype(f16)
        m["wo_s"] = np.ascontiguousarray(wo[QW * c:QW * (c + 1), :]).astype(f16)
        in_maps.append(m)
    return in_maps


def kernel(hidden_states, wq, wk, wv, wo, q_norm_w, k_norm_w,
           _trace=False, _return_results=False):
    from concourse import bass_utils

    hidden_states = np.asarray(hidden_states)
    wq, wk, wv, wo = (np.asarray(a) for a in (wq, wk, wv, wo))
    q_norm_w, k_norm_w = np.asarray(q_norm_w), np.asarray(k_norm_w)

    if "nc" not in _CACHE:
        _CACHE["nc"] = _build()
    nc = _CACHE["nc"]

    in_maps = _host_prep(hidden_states, wq, wk, wv, wo, q_norm_w, k_norm_w)
    res = bass_utils.run_bass_kernel_spmd(
        nc, in_maps, core_ids=list(range(NCORES)), trace=_trace)

    out = np.zeros((B, S, H), np.float32)
    for c in range(NCORES):
        out += res.results[c]["out_part"].astype(np.float32)
    if _return_results:
        return out, res
    return out
